# revision 1
# baseline (speedup 1.0000x reference)
"""Associative-embedding loss kernel for 8 Trainium2 NeuronCores.

Math: per image b, with tl[n,c] = pred[b,c,ty,tx] and br[n,c] = target[b,c,by,bx]
gathered at the N=128 match points:
  pull_b = sum_{n,c} (tl-br)^2 / (2N)
  s[n]   = 0.5 * sum_c (tl+br),  A[i,j] = s[i]-s[j]
  push_b = sum_{i!=j} relu(1-|A[i,j]|) / (N(N-1))
Using antisymmetry of A:  sum_{ij} relu(1-|A|) = sum|A+1| - sum|A|, and the
diagonal contributes exactly N, so push_b = (sum|A+1| - sum|A| - N)/(N(N-1)).

Strategy: data-parallel over B (8 images per core). Inputs are relaid out
channels-last on the host so each match point is one contiguous 16B gather.
The device kernel issues 16 indirect DMAs (one per image x corner, 128
descriptors each; HW consumes one index per partition per instruction and
descriptor generation costs a fixed ~1.1-1.3us per instruction on the
GPSIMD Q7, so 16 instructions is the floor). Per image: e = tl+br and -e
feed two PE transposes giving lhsT = [e^T; 0.5] and rhs = [0.5; -e^T];
one K=8 matmul contracts over channels and yields the full pairwise
matrix A[i,j] = s_i - s_j in PSUM directly. |A+1| accumulates on ACT in
parallel with |A| on DVE. Emission is software-pipelined (fronts one
image ahead of backs) so chains hide under later descriptor generation.
Each core returns [128, 24] partial sums folded on the host.
"""

import numpy as np

B, C, H, W, N = 64, 4, 256, 256, 128
M = 8            # cores
BL = B // M      # images per core
HW = H * W

_GRAPH = None


def _build_graph():
    import concourse.bass as bass
    import concourse.bacc as bacc
    import concourse.mybir as mybir
    from concourse.tile import TileContext

    f32 = mybir.dt.float32
    i32 = mybir.dt.int32
    Alu = mybir.AluOpType
    Act = mybir.ActivationFunctionType

    nc = bacc.Bacc()
    pt_d = nc.declare_dram_parameter("pt", [1024, 1024, C], f32, isOutput=False)
    idx_d = nc.declare_dram_parameter("idx", [N, 2 * BL], i32, isOutput=False)
    ident_d = nc.declare_dram_parameter("ident", [128, 128], f32, isOutput=False)
    out_d = nc.declare_dram_parameter("out", [N, 3 * BL], f32, isOutput=True)

    with TileContext(nc) as tc:
        with (
            tc.tile_pool(name="sb", bufs=1) as pool,
            tc.tile_pool(name="w", bufs=1) as wpool,
            tc.tile_pool(name="ps", bufs=2, space="PSUM") as psum,
        ):
            idx_t = pool.tile([N, 2 * BL], i32)
            nc.sync.dma_start(out=idx_t[:], in_=idx_d[:])
            ident = pool.tile([128, 128], f32)
            nc.sync.dma_start(out=ident[:], in_=ident_d[:])

            acc = pool.tile([N, 3 * BL], f32)

            # transpose sources: xea = [e | 0.5], xeb = [0.5 | -e]; the 0.5
            # halves come from the prefill, e / -e overwrite the other half
            xea = [wpool.tile([N, 2 * C], f32, name=f"xea{b}", tag=f"xea{b}") for b in range(BL)]
            xeb = [wpool.tile([N, 2 * C], f32, name=f"xeb{b}", tag=f"xeb{b}") for b in range(BL)]
            for b in range(BL):
                nc.vector.memset(xea[b][:], 0.5)
                nc.vector.memset(xeb[b][:], 0.5)

            # idx col 2b = tl of image b, col 2b+1 = br of image b
            g = [wpool.tile([N, 2 * C], f32, name=f"g{b}", tag=f"g{b}") for b in range(BL)]
            for b in range(BL):
                for half in range(2):
                    k = 2 * b + half
                    nc.gpsimd.indirect_dma_start(
                        out=g[b][:, C * half:C * (half + 1)], out_offset=None,
                        in_=pt_d[:],
                        in_offset=bass.IndirectOffsetOnAxis(
                            ap=idx_t[:, k:k + 1], axis=1),
                    )

            def image_front(b):
                tl = g[b][:, 0:C]
                br = g[b][:, C:2 * C]

                # pull: acc[:, 3b] = sum_c (tl-br)^2
                d = wpool.tile([N, C], f32, name=f"d{b}", tag=f"d{b}")
                nc.vector.tensor_sub(d[:], tl, br)
                d2 = wpool.tile([N, C], f32, name=f"d2{b}", tag=f"d2{b}")
                nc.vector.scalar_tensor_tensor(
                    out=d2[:], in0=d[:], scalar=0.0, in1=d[:],
                    op0=Alu.bypass, op1=Alu.mult,
                    accum_out=acc[:, 3 * b:3 * b + 1],
                )

                # xea cols 0:4 = e = tl+br; xeb cols 4:8 = -e
                nc.vector.tensor_add(xea[b][:, 0:C], tl, br)
                nc.vector.scalar_tensor_tensor(
                    out=xeb[b][:, C:2 * C], in0=tl, scalar=-1.0, in1=br,
                    op0=Alu.mult, op1=Alu.subtract)

                ta_ps = psum.tile([2 * C, 128], f32, name=f"ta{b}", tag="ta", bufs=3)
                tb_ps = psum.tile([2 * C, 128], f32, name=f"tb{b}", tag="tb", bufs=3)
                nc.tensor.transpose(out=ta_ps[:], in_=xea[b][:], identity=ident[:])
                nc.tensor.transpose(out=tb_ps[:], in_=xeb[b][:], identity=ident[:])
                return ta_ps, tb_ps

            def image_back(b, ta_ps, tb_ps):
                lt = wpool.tile([2 * C, 128], f32, name=f"lt{b}", tag=f"lt{b}")
                rs = wpool.tile([2 * C, 128], f32, name=f"rs{b}", tag=f"rs{b}")
                nc.vector.tensor_copy(lt[:], ta_ps[:])
                nc.vector.tensor_copy(rs[:], tb_ps[:])

                # A[i,j] = sum_c 0.5*e[i,c] - 0.5*e[j,c] = s_i - s_j
                a_ps = psum.tile([128, 128], f32, name=f"a{b}", tag="a", bufs=2)
                nc.tensor.matmul(out=a_ps[:], lhsT=lt[:], rhs=rs[:],
                                 start=True, stop=True)

                # acc[:, 3b+1] = sum_j |A+1|, acc[:, 3b+2] = sum_j |A|
                scr = wpool.tile([128, 128], f32, name=f"scr{b}", tag=f"scr{b}")
                nc.scalar.activation(
                    out=scr[:], in_=a_ps[:], func=Act.Abs, bias=1.0, scale=1.0,
                    accum_out=acc[:, 3 * b + 1:3 * b + 2],
                )
                nc.vector.tensor_reduce(
                    out=acc[:, 3 * b + 2:3 * b + 3], in_=a_ps[:],
                    axis=mybir.AxisListType.X, op=Alu.add,
                    apply_absolute_value=True,
                )

            frq = []
            for b in range(BL + 1):
                if b < BL:
                    frq.append(image_front(b))
                if b >= 1:
                    image_back(b - 1, *frq[b - 1])

            nc.sync.dma_start(out=out_d[:, 0:3 * (BL - 2)], in_=acc[:, 0:3 * (BL - 2)])
            nc.sync.dma_start(out=out_d[:, 3 * (BL - 2):], in_=acc[:, 3 * (BL - 2):])
    nc.finalize()
    return nc


def _get_graph():
    global _GRAPH
    if _GRAPH is None:
        _GRAPH = _build_graph()
    return _GRAPH


def _make_in_maps(pred, target, match):
    pred_cl = np.ascontiguousarray(np.transpose(pred, (0, 2, 3, 1)))
    targ_cl = np.ascontiguousarray(np.transpose(target, (0, 2, 3, 1)))
    ident = np.eye(128, dtype=np.float32)
    in_maps = []
    base = (np.arange(BL, dtype=np.int64) * HW)[:, None]
    for i in range(M):
        sl = slice(i * BL, (i + 1) * BL)
        m = match[sl].astype(np.int64)
        itl = base + m[:, :, 0, 0] * W + m[:, :, 0, 1]            # [BL, N]
        ibr = BL * HW + base + m[:, :, 1, 0] * W + m[:, :, 1, 1]  # [BL, N]
        # interleave: col 2b = tl_b, col 2b+1 = br_b
        idx = np.empty((N, 2 * BL), np.int32)
        idx[:, 0::2] = itl.T
        idx[:, 1::2] = ibr.T
        pt = np.concatenate(
            [pred_cl[sl].reshape(512, 1024, C), targ_cl[sl].reshape(512, 1024, C)],
            axis=0,
        )
        in_maps.append({"pt": pt, "idx": idx, "ident": ident})
    return in_maps


def _finish(core_outs):
    pull_total = 0.0
    m_total = 0.0
    for o in core_outs:
        o = o.astype(np.float64).reshape(N, BL, 3)
        pull_total += o[:, :, 0].sum()
        m_total += (o[:, :, 1] - o[:, :, 2]).sum()
    pull_all = 0.25 * pull_total / (2 * N)
    push_all = 0.25 * (m_total - B * N) / (N * (N - 1))
    return (np.float32(pull_all), np.float32(push_all))


def kernel(pred, target, match):
    from concourse.bass_utils import run_bass_kernel_spmd

    nc = _get_graph()
    in_maps = _make_in_maps(np.asarray(pred), np.asarray(target), np.asarray(match))
    res = run_bass_kernel_spmd(nc, in_maps, core_ids=list(range(M)))
    return _finish([r["out"] for r in res.results])



# revision 17
# speedup vs baseline: 2.4182x; 2.4182x over previous
"""Associative-embedding loss kernel for 8 Trainium2 NeuronCores.

Math: per image b, with tl[n,c] = pred[b,c,ty,tx] and br[n,c] = target[b,c,by,bx]
gathered at the N=128 match points:
  pull_b = sum_{n,c} (tl-br)^2 / (2N)
  s'[n]  = sum_c (tl+br),  A'[i,j] = s'[i]-s'[j]   (A = A'/2)
  push_b = (0.5*(sum|A'+2| - sum|A'|) - N) / (N(N-1))
using sum_{ij} relu(1-|A|) = sum|A+1| - sum|A| for antisymmetric A.

Strategy: data-parallel over B (8 images per core). The host shards each
core's 128x2 match points into three small uploads (~320KB/core, vs the
16MB relayout the on-device-gather variant staged); HW indirect DMA is
limited to one index per partition per instruction (~1.3us each, 16 per
core), which made the gather the dominant cost on-device.

All loss arithmetic runs on device. The corner/channel sums that produce
s' are folded into the pairwise matmul contraction (K=128, bf16):
  lhsT rows 8b+q       = raw values v[b, i, q]  (q = 8 corner x channel)
  lhsT rows 64+..      = -1
  rhs rows 8b+q        = 1 on column block b (constant indicator)
  rhs rows 64+8b+q     = v[b, j, q] on column block b, zeros elsewhere
  => out[i, 128b+j] = sum_q v[b,i,q] - sum_q v[b,j,q] = s'_b[i] - s'_b[j]
for all 8 images across two PSUM banks [128, 512]. The Scalar engine
accumulates |A'+2| (Abs with bias via accum_out), the Vector engine
row-reduces |A'|, and pull comes from an fp32 subtract + square-accumulate
on a separate [8, 1024] upload. bf16 rounding only perturbs s' by ~0.4%,
far inside the 2e-2 gate; pull stays fp32 exact.
Each core returns [128, 8] partial sums folded on the host in fp64.
"""

import numpy as np

B, C, H, W, N = 64, 4, 256, 256, 128
M = 8            # cores
BL = B // M      # images per core
Q = 2 * C        # corner x channel values per point

_GRAPH = None

# constant indicator rows: row 8b+q is 1 on column block b
_IND = np.repeat(np.kron(np.eye(8), np.ones((1, N))), Q, axis=0)


def _build_graph():
    import concourse.bacc as bacc
    import concourse.mybir as mybir
    from concourse.tile import TileContext

    f32 = mybir.dt.float32
    bf16 = mybir.dt.bfloat16
    Alu = mybir.AluOpType
    Act = mybir.ActivationFunctionType
    Axis = mybir.AxisListType

    nc = bacc.Bacc()
    lt_d = nc.declare_dram_parameter("lt", [128, 128], bf16, isOutput=False)
    rh_d = nc.declare_dram_parameter("rh", [128, 8 * N], bf16, isOutput=False)
    g_d = nc.declare_dram_parameter("g", [8, 8 * N], f32, isOutput=False)
    out_d = nc.declare_dram_parameter("out", [128, 8], f32, isOutput=True)

    with TileContext(nc) as tc:
        with (
            tc.tile_pool(name="sb", bufs=1) as pool,
            tc.tile_pool(name="ps", bufs=2, space="PSUM") as psum,
        ):
            ltt = pool.tile([128, 128], bf16)
            nc.sync.dma_start(out=ltt[:], in_=lt_d[:])
            rht = pool.tile([128, 8 * N], bf16)
            nc.sync.dma_start(out=rht[:], in_=rh_d[:])
            g = pool.tile([8, 8 * N], f32)
            nc.sync.dma_start(out=g[:], in_=g_d[:])

            acc = pool.tile([128, 8], f32)
            nc.vector.memset(acc[:], 0.0)
            two = pool.tile([128, 1], f32)
            nc.vector.memset(two[:], 2.0)

            # A'[i, 128b+j] = s'_b[i] - s'_b[j], 4 images per PSUM bank
            bankA = psum.tile([128, 512], f32, name="bankA", tag="a")
            bankB = psum.tile([128, 512], f32, name="bankB", tag="b")
            nc.tensor.matmul(out=bankA[:], lhsT=ltt[:], rhs=rht[:, 0:512],
                             start=True, stop=True)
            nc.tensor.matmul(out=bankB[:], lhsT=ltt[:], rhs=rht[:, 512:1024],
                             start=True, stop=True)

            # acc col1/2 = rowsum |A'+2|; col3/4 = rowsum |A'|
            scr = pool.tile([128, 512], f32)
            nc.scalar.activation(
                out=scr[:], in_=bankA[:], func=Act.Abs, bias=two[:, 0:1],
                scale=1.0, accum_out=acc[:, 1:2])
            nc.scalar.activation(
                out=scr[:], in_=bankB[:], func=Act.Abs, bias=two[:, 0:1],
                scale=1.0, accum_out=acc[:, 2:3])
            nc.vector.tensor_reduce(
                out=acc[:, 3:4], in_=bankA[:], axis=Axis.X, op=Alu.add,
                apply_absolute_value=True)
            nc.vector.tensor_reduce(
                out=acc[:, 4:5], in_=bankB[:], axis=Axis.X, op=Alu.add,
                apply_absolute_value=True)

            # pull: acc[b, 0] = sum (tl - br)^2 over image b (fp32)
            dt_ = pool.tile([8, 4 * N], f32)
            nc.vector.scalar_tensor_tensor(
                out=dt_[:], in0=g[:, 0:512], scalar=0.0,
                in1=g[:, 512:1024], op0=Alu.bypass, op1=Alu.subtract)
            d2 = pool.tile([8, 4 * N], f32)
            nc.vector.scalar_tensor_tensor(
                out=d2[:], in0=dt_[:], scalar=0.0, in1=dt_[:],
                op0=Alu.bypass, op1=Alu.mult, accum_out=acc[0:8, 0:1])

            nc.sync.dma_start(out=out_d[:], in_=acc[:])
    nc.finalize()
    return nc


def _get_graph():
    global _GRAPH
    if _GRAPH is None:
        _GRAPH = _build_graph()
    return _GRAPH


def _make_in_maps(pred, target, match):
    import ml_dtypes

    bf16 = ml_dtypes.bfloat16
    barr = np.arange(B)[:, None]
    tl = pred[barr, :, match[:, :, 0, 0], match[:, :, 0, 1]]    # [B, N, C]
    br = target[barr, :, match[:, :, 1, 0], match[:, :, 1, 1]]  # [B, N, C]
    raw = np.concatenate([tl, br], axis=-1)                     # [B, N, Q]
    raw16 = raw.astype(bf16)

    in_maps = []
    for i in range(M):
        sl = slice(i * BL, (i + 1) * BL)
        rc = raw16[sl]                                          # [BL, N, Q]
        lt = np.empty((128, 128), bf16)
        lt[0:64] = rc.transpose(0, 2, 1).reshape(64, N)         # rows 8b+q
        lt[64:128] = bf16(-1.0)
        rh = np.zeros((128, 8 * N), bf16)
        rh[0:64] = _IND
        for b in range(BL):
            rh[64 + Q * b:64 + Q * (b + 1), N * b:N * (b + 1)] = \
                rc[b].transpose(1, 0)
        g = np.empty((8, 8 * N), np.float32)
        g[:, 0:512] = tl[sl].reshape(BL, 4 * N)
        g[:, 512:1024] = br[sl].reshape(BL, 4 * N)
        in_maps.append({"lt": lt, "rh": rh, "g": g})
    return in_maps


def _finish(core_outs):
    pull_total = 0.0
    m_total = 0.0
    for o in core_outs:
        o = np.asarray(o, dtype=np.float64)
        pull_total += o[:, 0].sum()
        m_total += (o[:, 1] + o[:, 2] - o[:, 3] - o[:, 4]).sum()
    # per image: 0.5*(sum|A'+2| - sum|A'|) = P_b + N
    pull_all = 0.25 * pull_total / (2 * N)
    push_all = 0.25 * (0.5 * m_total - B * N) / (N * (N - 1))
    return (np.float32(pull_all), np.float32(push_all))


def kernel(pred, target, match):
    from concourse.bass_utils import run_bass_kernel_spmd

    nc = _get_graph()
    in_maps = _make_in_maps(np.asarray(pred), np.asarray(target), np.asarray(match))
    res = run_bass_kernel_spmd(nc, in_maps, core_ids=list(range(M)))
    return _finish([r["out"] for r in res.results])
